# revision 2
# baseline (speedup 1.0000x reference)
"""Trainium2 Bass kernel for nn_CrossAttention (B=2, Lq=Lkv=2048, E=1024, H=16, D=64).

Head-sharded (2 heads/core), bf16 datapath, with:
  - host-side mask packing: masked-out kv positions are dropped before the
    device sees them (exact; softmax over the surviving set is identical),
    KC=1280 capacity vs 2048 raw.
  - K/V projections share one streamed x tile (half the input DMA).
  - context matmuls contract all 128 k-rows at once; a ones column in the
    stationary makes each also emit the softmax denominator in PSUM.
  - software-pipelined tile loop: scores(kc) issue ahead of ctx(kc-1) so the
    exp (scalar engine) hides under PE work; output-projection and the next
    tile's Q-projection matmuls are interleaved into the attention loop.
  - output partials in bf16; host sums the 8 partials and adds bo.
"""

import sys

if "/opt/trn_rl_repo" not in sys.path:
    sys.path.insert(0, "/opt/trn_rl_repo")

import numpy as np
import ml_dtypes

import concourse.tile as tile
from concourse import bacc, mybir
from concourse.bass_utils import run_bass_kernel_spmd
from concourse.masks import make_identity

F32 = mybir.dt.float32
BF16 = mybir.dt.bfloat16
AF = mybir.ActivationFunctionType
BF = ml_dtypes.bfloat16

N_CORES = 8
B, LQ, LKV, E, H, D = 2, 2048, 2048, 1024, 16, 64
HC = H // N_CORES  # 2 heads per core
JC = HC * D  # 128
T = B * LQ  # 4096
NEC = E // 128  # 8
NOC = E // 128  # 8
NQT = LQ // 512  # 4
NTT = B * NQT  # 8

_NC_CACHE = {}


def build(reps=None, KC=1280):
    key = (reps or 0, KC)
    if key in _NC_CACHE:
        return _NC_CACHE[key]
    NKT = KC // 128  # k chunks per batch
    TKV = B * KC
    NKVT = TKV // 512  # kv projection tiles

    nc = bacc.Bacc("TRN2", target_bir_lowering=False, debug=False, num_devices=N_CORES)

    xqT = nc.dram_tensor("xqT", [E, T], BF16, kind="ExternalInput").ap()
    xkT = nc.dram_tensor("xkT", [E, TKV], BF16, kind="ExternalInput").ap()
    wqT = nc.dram_tensor("wqT", [E, JC], BF16, kind="ExternalInput").ap()
    wkT = nc.dram_tensor("wkT", [E, JC], BF16, kind="ExternalInput").ap()
    wvT = nc.dram_tensor("wvT", [E, JC], BF16, kind="ExternalInput").ap()
    woT = nc.dram_tensor("woT", [JC, E], BF16, kind="ExternalInput").ap()
    bqd = nc.dram_tensor("bq", [JC, 1], F32, kind="ExternalInput").ap()
    bkd = nc.dram_tensor("bk", [JC, 1], F32, kind="ExternalInput").ap()
    bvd = nc.dram_tensor("bv", [JC, 1], F32, kind="ExternalInput").ap()
    mbd = nc.dram_tensor("mb", [B, NKT, 128], F32, kind="ExternalInput").ap()
    outT = nc.dram_tensor("outT", [E, T], BF16, kind="ExternalOutput").ap()

    from contextlib import nullcontext

    with tile.TileContext(nc) as tc, nc.allow_low_precision(reason="bf16 kernel"):
        with tc.For_i(0, reps, 1) if reps else nullcontext():
         with (
             tc.tile_pool(name="const", bufs=1) as const,
             tc.tile_pool(name="big", bufs=1) as big,
         ):
            wq_sb = const.tile([128, NEC, JC], BF16, tag="wq")
            nc.sync.dma_start(out=wq_sb, in_=wqT.rearrange("(ec p) j -> p ec j", p=128))
            wk_sb = const.tile([128, NEC, JC], BF16, tag="wk")
            nc.sync.dma_start(out=wk_sb, in_=wkT.rearrange("(ec p) j -> p ec j", p=128))
            wv_sb = const.tile([128, NEC, JC], BF16, tag="wv")
            nc.sync.dma_start(out=wv_sb, in_=wvT.rearrange("(ec p) j -> p ec j", p=128))
            wo_sb = const.tile([128, NOC, 128], BF16, tag="wo")
            nc.sync.dma_start(out=wo_sb, in_=woT.rearrange("p (oc o) -> p oc o", oc=NOC))
            bq_sb = const.tile([128, 1], F32, tag="bq")
            nc.sync.dma_start(out=bq_sb, in_=bqd)
            bk_sb = const.tile([128, 1], F32, tag="bk")
            nc.sync.dma_start(out=bk_sb, in_=bkd)
            bv_sb = const.tile([128, 1], F32, tag="bv")
            nc.sync.dma_start(out=bv_sb, in_=bvd)
            mb_sb = const.tile([128, B, NKT], F32, tag="mb")
            nc.sync.dma_start(out=mb_sb, in_=mbd.rearrange("b kc p -> p b kc"))
            ident = const.tile([128, 128], BF16, tag="ident")
            make_identity(nc, ident)
            onesP = const.tile([128, 65], BF16, tag="onesP")
            nc.vector.memset(onesP, 1.0)

            kt_sb = big.tile([128, TKV], BF16, tag="kt")
            vt_sb = big.tile([128, TKV], BF16, tag="vt")
            v_sb = big.tile([128, B * NKT, 130], BF16, tag="v")

            # ---- phase KV: K/V projections from one streamed x tile ----
            with (
                tc.tile_pool(name="xkv", bufs=2) as xkv,
                tc.tile_pool(name="kvp", bufs=2, space="PSUM") as kvp,
            ):
                for i in range(NKVT):
                    xt = xkv.tile([128, NEC, 512], BF16, tag="xkv")
                    nc.sync.dma_start(
                        out=xt,
                        in_=xkT[:, i * 512 : (i + 1) * 512].rearrange(
                            "(ec p) t -> p ec t", p=128
                        ),
                    )
                    for wsb, bias, dst in (
                        (wk_sb, bk_sb, kt_sb),
                        (wv_sb, bv_sb, vt_sb),
                    ):
                        pt = kvp.tile([128, 512], F32, tag="kvp")
                        for ec in range(NEC):
                            nc.tensor.matmul(
                                pt, wsb[:, ec, :], xt[:, ec, :],
                                start=(ec == 0), stop=(ec == NEC - 1),
                            )
                        nc.scalar.activation(
                            out=dst[:, i * 512 : (i + 1) * 512],
                            in_=pt, func=AF.Identity, bias=bias, scale=1.0,
                        )

            # ---- phase T: V^T -> v_sb [k, gc, [Vh0|1|1|Vh1]] ----
            with tc.tile_pool(name="tp", bufs=3, space="PSUM") as tp:
                nc.vector.memset(v_sb[:, :, 64:66], 1.0)
                for gc in range(B * NKT):
                    tpt = tp.tile([128, 128], BF16, tag="tp")
                    nc.tensor.transpose(
                        tpt, vt_sb[:, gc * 128 : (gc + 1) * 128], ident
                    )
                    nc.vector.tensor_copy(v_sb[:, gc, 0:64], tpt[:, 0:64])
                    nc.vector.tensor_copy(v_sb[:, gc, 66:130], tpt[:, 64:128])

            # ---- phase QAO: pipelined per-512-token tile ----
            with (
                tc.tile_pool(name="xq", bufs=3) as xqp,
                tc.tile_pool(name="qt", bufs=3) as qtp,
                tc.tile_pool(name="emt", bufs=3) as emtp,
                tc.tile_pool(name="ctx", bufs=2) as ctxp,
                tc.tile_pool(name="cs1", bufs=2) as cs1p,
                tc.tile_pool(name="rr", bufs=2) as rrp,
                tc.tile_pool(name="outsb", bufs=2) as outp,
                tc.tile_pool(name="ps2", bufs=2, space="PSUM") as ps2,
                tc.tile_pool(name="ps1", bufs=1, space="PSUM") as ps1,
            ):
                state = {}

                def emit_dma_xq(tt):
                    xt = xqp.tile([128, NEC, 512], BF16, tag="xq", name=f"xq_{tt}")
                    nc.sync.dma_start(
                        out=xt,
                        in_=xqT[:, tt * 512 : (tt + 1) * 512].rearrange(
                            "(ec p) t -> p ec t", p=128
                        ),
                    )
                    state[("xq", tt)] = xt

                def emit_qproj_mm(tt, ec):
                    if ec == 0:
                        state[("qp", tt)] = ps1.tile(
                            [128, 512], F32, tag="aux", name=f"qp_{tt}"
                        )
                    nc.tensor.matmul(
                        state[("qp", tt)], wq_sb[:, ec, :],
                        state[("xq", tt)][:, ec, :],
                        start=(ec == 0), stop=(ec == NEC - 1),
                    )

                def emit_qproj_act(tt):
                    qt = qtp.tile([128, 512], BF16, tag="qt", name=f"qt_{tt}")
                    nc.scalar.activation(
                        out=qt, in_=state[("qp", tt)],
                        func=AF.Identity, bias=bq_sb, scale=1.0,
                    )
                    state[("qt", tt)] = qt

                def emit_epi0(tt):
                    # head0: denom at cx0 row 64; ctx rows 0-63
                    rr = rrp.tile([65, 512], BF16, tag="rr", name=f"rr_{tt}")
                    state[("rr", tt)] = rr
                    cx0, cx1 = state[("cx", tt)]
                    nc.vector.reciprocal(rr[64:65, :], cx0[64:65, :])
                    nc.vector.reciprocal(rr[0:1, :], cx1[0:1, :])

                def emit_epi1(tt):
                    cx0, _ = state[("cx", tt)]
                    rr = state[("rr", tt)]
                    bt = ps1.tile([128, 512], F32, tag="aux", name=f"bct0_{tt}")
                    nc.tensor.matmul(
                        bt[0:65, :], onesP[64:65, :], rr[64:65, :],
                        start=True, stop=True,
                    )
                    s0 = cs1p.tile([65, 512], BF16, tag="s0", name=f"s0_{tt}")
                    nc.vector.tensor_copy(s0, cx0)
                    ctx = ctxp.tile([128, 512], BF16, tag="ctx", name=f"ctx_{tt}")
                    state[("ctx", tt)] = ctx
                    nc.vector.tensor_mul(ctx[0:64, :], s0[0:64, :], bt[0:64, :])

                def emit_epi2(tt):
                    # head1: denom at cx1 row 0; ctx rows 1-64 -> shift via DMA
                    _, cx1 = state[("cx", tt)]
                    rr = state[("rr", tt)]
                    bt = ps1.tile([128, 512], F32, tag="aux", name=f"bct1_{tt}")
                    nc.tensor.matmul(
                        bt[0:65, :], onesP[0:1, :], rr[0:1, :],
                        start=True, stop=True,
                    )
                    s1 = cs1p.tile([65, 512], BF16, tag="s1", name=f"s1_{tt}")
                    nc.vector.tensor_copy(s1, cx1)
                    cs = cs1p.tile([65, 512], BF16, tag="cs1", name=f"cs1_{tt}")
                    nc.vector.tensor_mul(cs, s1, bt[0:65, :])
                    nc.sync.dma_start(
                        out=state[("ctx", tt)][64:128, :], in_=cs[1:65, :]
                    )

                def emit_omm(tt, oc):
                    if oc == 0:
                        state[("ob", tt)] = outp.tile(
                            [128, NOC, 512], BF16, tag="ob", name=f"ob_{tt}"
                        )
                    op = ps1.tile([128, 512], F32, tag="op", name=f"op_{tt}_{oc}")
                    nc.tensor.matmul(
                        op, wo_sb[:, oc, :], state[("ctx", tt)],
                        start=True, stop=True,
                    )
                    nc.vector.tensor_copy(state[("ob", tt)][:, oc, :], op)

                def emit_outdma(tt):
                    nc.sync.dma_start(
                        out=outT[:, tt * 512 : (tt + 1) * 512].rearrange(
                            "(oc p) t -> p oc t", p=128
                        ),
                        in_=state[("ob", tt)],
                    )
                    del state[("ob", tt)], state[("ctx", tt)]

                emit_dma_xq(0)
                emit_dma_xq(1)
                for ec in range(NEC):
                    emit_qproj_mm(0, ec)
                emit_qproj_act(0)

                for tt in range(NTT):
                    b = tt // NQT
                    if tt + 2 < NTT:
                        emit_dma_xq(tt + 2)
                    cx0 = ps1.tile([65, 512], F32, tag="cx0", name=f"cx0_{tt}")
                    cx1 = ps1.tile([65, 512], F32, tag="cx1", name=f"cx1_{tt}")
                    state[("cx", tt)] = (cx0, cx1)
                    qt = state[("qt", tt)]
                    for kc in range(NKT + 1):
                        if kc < NKT:
                            k0 = (b * NKT + kc) * 128
                            sct = ps2.tile(
                                [128, 2, 512], F32, tag="sct", name=f"sct_{tt}_{kc}"
                            )
                            nc.tensor.matmul(
                                sct[:, 0, :], kt_sb[0:64, k0 : k0 + 128],
                                qt[0:64, :], start=True, stop=True,
                            )
                            nc.tensor.matmul(
                                sct[:, 1, :], kt_sb[64:128, k0 : k0 + 128],
                                qt[64:128, :], start=True, stop=True,
                            )
                            emt = emtp.tile(
                                [128, 2, 512], BF16, tag="emt", name=f"emt_{tt}_{kc}"
                            )
                            nc.scalar.activation(
                                out=emt.rearrange("p a t -> p (a t)"),
                                in_=sct.rearrange("p a t -> p (a t)"),
                                func=AF.Exp,
                                bias=mb_sb[:, b, kc : kc + 1],
                                scale=0.125,
                            )
                            state[("emt", kc)] = emt
                        # interleaved extras: prev tile epilogue+O, next tile Q
                        if kc == 0 and tt > 0:
                            emit_epi1(tt - 1)
                        elif kc == 1 and tt > 0:
                            emit_epi2(tt - 1)
                        elif 2 <= kc < 2 + NOC and tt > 0:
                            emit_omm(tt - 1, kc - 2)
                        if 2 <= kc < 2 + NEC and tt + 1 < NTT:
                            emit_qproj_mm(tt + 1, kc - 2)
                        if kc >= 1:
                            kp = kc - 1
                            gc = b * NKT + kp
                            emp = state[("emt", kp)]
                            st, sp = (kp == 0), (kp == NKT - 1)
                            nc.tensor.matmul(
                                cx0, v_sb[:, gc, 0:65], emp[:, 0, :],
                                start=st, stop=sp,
                            )
                            nc.tensor.matmul(
                                cx1, v_sb[:, gc, 65:130], emp[:, 1, :],
                                start=st, stop=sp,
                            )
                    if tt + 1 < NTT:
                        emit_qproj_act(tt + 1)
                    emit_epi0(tt)
                    if tt > 0:
                        emit_outdma(tt - 1)

                tt = NTT - 1
                emit_epi1(tt)
                emit_epi2(tt)
                for oc in range(NOC):
                    emit_omm(tt, oc)
                emit_outdma(tt)

    nc.compile()
    _NC_CACHE[key] = nc
    return nc


def _pick_kc(mask):
    mx = max(int((mask[b] != 0).sum()) for b in range(B))
    for kc in (1280, 1536, 1792, 2048):
        if mx <= kc:
            return kc
    return 2048


def make_in_maps(query, key_value, mask, Wq, bq, Wk, bk, Wv, bv, Wo, bo, KC=1280):
    NKT = KC // 128
    xqT = np.ascontiguousarray(
        np.asarray(query, np.float32).reshape(T, E).T
    ).astype(BF)
    kvp = np.zeros((B, KC, E), np.float32)
    mbias = np.full((B, KC), -1.0e5, np.float32)
    kv = np.asarray(key_value, np.float32)
    for b in range(B):
        idx = np.nonzero(np.asarray(mask)[b] != 0)[0]
        n = min(len(idx), KC)
        kvp[b, :n] = kv[b][idx[:n]]
        mbias[b, :n] = 0.0
    xkT = np.ascontiguousarray(kvp.reshape(B * KC, E).T).astype(BF)
    mb = mbias.reshape(B, NKT, 128)
    in_maps = []
    for c in range(N_CORES):
        sl = slice(c * JC, (c + 1) * JC)
        in_maps.append(
            {
                "xqT": xqT,
                "xkT": xkT,
                "wqT": np.ascontiguousarray(np.asarray(Wq)[sl, :].T).astype(BF),
                "wkT": np.ascontiguousarray(np.asarray(Wk)[sl, :].T).astype(BF),
                "wvT": np.ascontiguousarray(np.asarray(Wv)[sl, :].T).astype(BF),
                "woT": np.ascontiguousarray(np.asarray(Wo)[:, sl].T).astype(BF),
                "bq": np.asarray(bq)[sl].reshape(JC, 1).astype(np.float32),
                "bk": np.asarray(bk)[sl].reshape(JC, 1).astype(np.float32),
                "bv": np.asarray(bv)[sl].reshape(JC, 1).astype(np.float32),
                "mb": mb,
            }
        )
    return in_maps


def kernel(query, key_value, mask, Wq, bq, Wk, bk, Wv, bv, Wo, bo):
    KC = _pick_kc(np.asarray(mask))
    nc = build(None, KC)
    in_maps = make_in_maps(
        query, key_value, mask, Wq, bq, Wk, bk, Wv, bv, Wo, bo, KC=KC
    )
    res = run_bass_kernel_spmd(nc, in_maps, list(range(N_CORES)))
    acc = np.zeros((E, T), np.float32)
    for c in range(N_CORES):
        acc += np.asarray(res.results[c]["outT"], dtype=np.float32)
    acc += np.asarray(bo, np.float32).reshape(E, 1)
    return np.ascontiguousarray(acc.T).reshape(B, LQ, E).astype(np.float32)
